# revision 14
# baseline (speedup 1.0000x reference)
"""Character-delimited (segment-local causal) attention on 8 trn2 cores.

Sharding: core = (batch, head-half): b = core//2, hh = core%2.
Each core computes the qkv projection for its batch restricted to its 8
heads (512 of the 3072 Wqkv columns per section) plus the segment-sparse
attention for those heads.

Device pipeline (per core, bf16 matmul operands / fp32 accumulation):
  - x is loaded TRANSPOSED via xbar DMA-transpose (bf16) as xT tiles.
  - q,k are produced transposed (qT/kT: [d, s]) so QK^T needs no further
    transposes; v is produced in natural [s, d] layout with an extra
    all-ones column per head so the PV matmul also accumulates the softmax
    denominator (row 64 of the ctx psum).
  - Attention is block-sparse: for each 512-query chunk only key blocks
    that can contain same-segment keys are computed (block geometry is
    derived on the host from char_ids and baked into the program; per-batch
    exactness is restored by elementwise 0/1 masks multiplied into
    exp(scores)).  scores are computed transposed ([k, q]) so exp(scores)
    feeds the PV matmul directly as the moving operand.
  - ctx^T [65, 512] (64 dims + denominator row) is transposed back on the
    PE (fp32) and the division by the denominator is folded into the
    PSUM->SBUF copy via a per-partition reciprocal scale.
"""

import numpy as np
import ml_dtypes

B, S, E = 4, 2048, 1024
H, D = 16, 64
NCORES = 8
CH = 512          # query chunk
KB = 128          # key block
NCH = S // CH     # 4 chunks
DELIMS = (32, 10)
HPC = H // 2      # heads per core (8)

_prog_cache = {}


def _segments(char_ids):
    """seg ids, per-position segment start / end (exclusive), per batch."""
    ids = np.asarray(char_ids)
    is_d = np.zeros(ids.shape, dtype=bool)
    for d in DELIMS:
        is_d |= ids == d
    seg = np.cumsum(is_d.astype(np.int64), axis=-1)
    starts = np.empty_like(seg)
    ends = np.empty_like(seg)
    for b in range(seg.shape[0]):
        starts[b] = np.searchsorted(seg[b], seg[b], side="left")
        ends[b] = np.searchsorted(seg[b], seg[b], side="right")
    return seg, starts, ends


def _geometry(seg, starts, ends):
    """Shared (union over batches) block geometry.

    Returns blocks[qc] = list of (k0, qoff, N, moff) and mask width MASKC.
    """
    blocks = []
    maskc = 0
    for qc in range(NCH):
        q0 = qc * CH
        sstart_min = int(starts[:, q0].min())
        send_max = int(ends[:, q0].max())
        past_lo = (sstart_min // KB) * KB
        qe_past = min(send_max - q0, CH)
        blist = []
        moff = 0
        for k0 in range(past_lo, q0, KB):
            blist.append((k0, 0, qe_past, moff))
            moff += qe_past
        for kc in range(CH // KB):
            k0 = q0 + kc * KB
            de = int(ends[:, k0 + KB - 1].max())
            de = min(max(de, k0 + KB), q0 + CH)
            n = de - k0
            blist.append((k0, kc * KB, n, moff))
            moff += n
        blocks.append(blist)
        maskc = max(maskc, moff)
    return blocks, maskc


def _masks_for_batch(seg_b, blocks, maskc):
    """[NCH, 128, maskc] bfloat16 0/1 mask blob for one batch."""
    out = np.zeros((NCH, KB, maskc), dtype=ml_dtypes.bfloat16)
    pos = np.arange(S)
    for qc, blist in enumerate(blocks):
        q0 = qc * CH
        for (k0, qoff, n, moff) in blist:
            kk = pos[k0:k0 + KB]
            qq = pos[q0 + qoff:q0 + qoff + n]
            m = (seg_b[kk][:, None] == seg_b[qq][None, :]) & (kk[:, None] <= qq[None, :])
            out[qc, :, moff:moff + n] = m.astype(ml_dtypes.bfloat16)
    return out


def _group_blocks(blist):
    """Pack consecutive blocks into groups whose total q-extent fits one
    512-col psum bank.  Returns [(g_moff, gN, [(k0, qoff, n, moff), ...])]."""
    groups = []
    cur, width = [], 0
    for blk in blist:
        n = blk[2]
        if cur and width + n > CH:
            groups.append((cur[0][3], width, cur))
            cur, width = [], 0
        cur.append(blk)
        width += n
    if cur:
        groups.append((cur[0][3], width, cur))
    return groups


def _build_program(blocks, maskc):
    import concourse.bacc as bacc
    import concourse.tile as tile
    from concourse import mybir
    from contextlib import ExitStack

    f32 = mybir.dt.float32
    bf16 = mybir.dt.bfloat16
    AF = mybir.ActivationFunctionType

    nc = bacc.Bacc("TRN2", target_bir_lowering=False, debug=False,
                   num_devices=NCORES)

    x_h = nc.dram_tensor("x", [S, E], bf16, kind="ExternalInput")
    w_h = nc.dram_tensor("w", [E, 3 * CH], bf16, kind="ExternalInput")
    bqk_h = nc.dram_tensor("bqk", [128, 8], f32, kind="ExternalInput")
    bv_h = nc.dram_tensor("bv", [1, CH], bf16, kind="ExternalInput")
    mk_h = nc.dram_tensor("masks", [NCH, KB, maskc], bf16, kind="ExternalInput")
    id_h = nc.dram_tensor("ident", [128, 128], f32, kind="ExternalInput")
    out_h = nc.dram_tensor("out", [S, CH], f32, kind="ExternalOutput")

    ET = E // 128   # 8 e-tiles

    with tile.TileContext(nc) as tc:
        with ExitStack() as ctx:
            sing = ctx.enter_context(tc.tile_pool(name="sing", bufs=1))
            xtp = ctx.enter_context(tc.tile_pool(name="xtp", bufs=12))
            qp = ctx.enter_context(tc.tile_pool(name="qp", bufs=2))
            mp = ctx.enter_context(tc.tile_pool(name="mp", bufs=2))
            esp = ctx.enter_context(tc.tile_pool(name="esp", bufs=7))
            ctsp = ctx.enter_context(tc.tile_pool(name="ctsp", bufs=3))
            recp = ctx.enter_context(tc.tile_pool(name="recp", bufs=3))
            outp = ctx.enter_context(tc.tile_pool(name="outp", bufs=2))

            ph1 = ctx.enter_context(tc.tile_pool(name="ph1", bufs=2, space="PSUM"))
            scrp = ctx.enter_context(tc.tile_pool(name="scrp", bufs=3, space="PSUM"))
            ctxp = ctx.enter_context(tc.tile_pool(name="ctxp", bufs=3, space="PSUM"))

            # ---- constants / persistent tensors ----
            w_sb = sing.tile([128, ET, 3 * CH], bf16, tag="w")
            for et in range(ET):
                nc.sync.dma_start(out=w_sb[:, et, :],
                                  in_=w_h[et * 128:(et + 1) * 128, :])
            id_sb = sing.tile([128, 128], f32, tag="ident")
            nc.sync.dma_start(out=id_sb, in_=id_h[:, :])
            bqk_sb = sing.tile([128, 8], f32, tag="bqk")
            nc.sync.dma_start(out=bqk_sb, in_=bqk_h[:, :])
            bv_sb = sing.tile([1, CH], bf16, tag="bv")
            nc.sync.dma_start(out=bv_sb, in_=bv_h[:, :])
            ones_sb = sing.tile([1, 128], bf16, tag="ones")
            nc.vector.memset(ones_sb, 1.0)
            zcol_sb = sing.tile([1, 65], bf16, tag="zcol")
            nc.vector.memset(zcol_sb, 0.0)

            k_sbs, v_sbs = [], []
            for c in range(NCH):
                kt_ = sing.tile([128, 4, CH], bf16, tag=f"k{c}")
                vt_ = sing.tile([128, 4, HPC, 65], bf16, tag=f"v{c}")
                nc.vector.memset(vt_[:, :, :, 64:65], 1.0)
                k_sbs.append(kt_)
                v_sbs.append(vt_)

            # -------- software-pipelined emission: phase1(sc) ∥ attn(sc-1) ----
            q_tiles = {}
            mask_tiles = {}

            def phase1_units(sc):
                units = []

                def load(sc=sc):
                    xts = []
                    if sc == 0:
                        # chunk 0: PE transposes (x arrives via fast plain
                        # DMAs; the PE is idle at startup anyway) to avoid
                        # the serialized xbar-transpose descriptor latency
                        xrows = []
                        for ss in range(4):
                            xr = xtp.tile([128, E], bf16, tag="xr", name="xr")
                            nc.sync.dma_start(
                                out=xr, in_=x_h[ss * 128:(ss + 1) * 128, :])
                            xrows.append(xr)
                        idb = sing.tile([128, 128], bf16, tag="idb")
                        nc.vector.tensor_copy(idb, id_sb)
                        for et in range(ET):
                            pt = scrp.tile([128, CH], bf16, tag="scr",
                                           name="pt")
                            for ss in range(4):
                                nc.tensor.transpose(
                                    pt[:, ss * 128:(ss + 1) * 128],
                                    xrows[ss][:, et * 128:(et + 1) * 128],
                                    idb)
                            xt_t = xtp.tile([128, CH], bf16, tag="xt")
                            if et % 2 == 0:
                                nc.scalar.copy(xt_t, pt)
                            else:
                                nc.vector.tensor_copy(xt_t, pt)
                            xts.append(xt_t)
                    else:
                        for et in range(ET):
                            xt_t = xtp.tile([128, CH], bf16, tag="xt")
                            nc.sync.dma_start(
                                out=xt_t,
                                in_=x_h[sc * CH:(sc + 1) * CH,
                                        et * 128:(et + 1) * 128],
                                transpose=True)
                            xts.append(xt_t)
                    phase1_units.xts = xts
                    mask_t = mp.tile([128, maskc], bf16, tag="m")
                    nc.sync.dma_start(out=mask_t, in_=mk_h[sc, :, :])
                    mask_tiles[sc] = mask_t
                    q_tiles[sc] = qp.tile([128, 4, CH], bf16, tag="q", name="q_t")
                units.append(load)

                def qk_unit(ot, sc=sc):
                    xts = phase1_units.xts
                    pq = ph1.tile([128, CH], f32, tag="ph1")
                    for et in range(ET):
                        nc.tensor.matmul(
                            pq, w_sb[:, et, ot * 128:(ot + 1) * 128], xts[et],
                            start=(et == 0), stop=(et == ET - 1))
                    if ot < 4:
                        nc.scalar.add(q_tiles[sc][:, ot, :], pq,
                                      bqk_sb[:, ot:ot + 1])
                    else:
                        nc.vector.tensor_scalar_add(k_sbs[sc][:, ot - 4, :],
                                                    pq, bqk_sb[:, ot:ot + 1])
                for ot in range(8):
                    units.append(lambda ot=ot: qk_unit(ot))

                def v_unit(ss, sc=sc):
                    xts = phase1_units.xts
                    pv = ph1.tile([128, CH], f32, tag="ph1")
                    for et in range(ET):
                        nc.tensor.matmul(
                            pv, xts[et][:, ss * 128:(ss + 1) * 128],
                            w_sb[:, et, 2 * CH:3 * CH],
                            start=(et == 0), stop=False)
                    nc.tensor.matmul(pv, ones_sb, bv_sb, start=False, stop=True)
                    nc.vector.tensor_copy(
                        v_sbs[sc][:, ss, :, 0:64],
                        pv.rearrange("p (h c) -> p h c", c=64))
                for ss in range(4):
                    units.append(lambda ss=ss: v_unit(ss))
                return units

            def attn_units(qc):
                groups = _group_blocks(blocks[qc])
                nb = len(blocks[qc])
                state = {}

                def start(qc=qc):
                    state["ost"] = outp.tile([128, 4, CH], f32, tag="ost", name="ost")

                def head_a(h, qc=qc):
                    q_t = q_tiles[qc]
                    mask_t = mask_tiles[qc]
                    p0 = (h % 2) * 64
                    ctx_t = ctxp.tile([65, CH], f32, tag="ct", name="ctx_t")
                    nc.tensor.matmul(ctx_t, zcol_sb, bv_sb,
                                     start=True, stop=False)
                    scrs = []
                    for gi2, (gm, gn, blks) in enumerate(groups):
                        if qc == NCH - 1 and gi2 % 2 == 0:
                            scr = ph1.tile([128, CH], f32, tag="ph1", name="scr")
                        else:
                            scr = scrp.tile([128, CH], f32, tag="scr")
                        for (k0, qoff, n, moff) in blks:
                            kci, koff = k0 // CH, k0 % CH
                            nc.tensor.matmul(
                                scr[:, moff - gm:moff - gm + n],
                                k_sbs[kci][p0:p0 + 64, h // 2,
                                           koff:koff + 128],
                                q_t[p0:p0 + 64, h // 2, qoff:qoff + n],
                                start=True, stop=True)
                        scrs.append(scr)
                    ess = []
                    for gi, (gm, gn, blks) in enumerate(groups):
                        es = esp.tile([128, CH], bf16, tag="es")
                        nc.scalar.activation(es[:, 0:gn], scrs[gi][:, 0:gn],
                                             AF.Exp)
                        nc.vector.tensor_mul(es[:, 0:gn], es[:, 0:gn],
                                             mask_t[:, gm:gm + gn])
                        ess.append(es)
                    state[("h", h)] = (ctx_t, ess)

                def head_b(h, qc=qc):
                    ost = state["ost"]
                    ctx_t, ess = state.pop(("h", h))
                    nb = len(blocks[qc])
                    ib = 0
                    for gi, (gm, gn, blks) in enumerate(groups):
                        for (k0, qoff, n, moff) in blks:
                            kci, koff = k0 // CH, k0 % CH
                            nc.tensor.matmul(
                                ctx_t[:, qoff:qoff + n],
                                v_sbs[kci][:, koff // 128, h, :],
                                ess[gi][:, moff - gm:moff - gm + n],
                                start=False, stop=(ib == nb - 1))
                            ib += 1

                    cts = ctsp.tile([65, CH], f32, tag="cts")
                    if h % 2 == 0:
                        nc.vector.tensor_copy(cts, ctx_t)
                    else:
                        nc.scalar.copy(cts, ctx_t)
                    tp = ctxp.tile([128, 4 * 65], f32, tag="ct", name="tp")
                    for i in range(4):
                        nc.tensor.transpose(
                            tp[:, i * 65:(i + 1) * 65],
                            cts[:, i * 128:(i + 1) * 128],
                            id_sb[0:65, 0:65])
                    rec = recp.tile([128, 4, 1], f32, tag="rec")
                    nc.vector.reciprocal(
                        rec,
                        tp.rearrange("p (i c) -> p i c", c=65)[:, :, 64:65])
                    for i in range(4):
                        if h % 2 == 0:
                            nc.scalar.mul(ost[:, i, h * 64:(h + 1) * 64],
                                          tp[:, i * 65:i * 65 + 64],
                                          rec[:, i:i + 1, 0])
                        else:
                            nc.vector.tensor_scalar_mul(
                                ost[:, i, h * 64:(h + 1) * 64],
                                tp[:, i * 65:i * 65 + 64],
                                rec[:, i:i + 1, 0])

                def finish(qc=qc):
                    nc.sync.dma_start(
                        out=out_h[qc * CH:(qc + 1) * CH, :].rearrange(
                            "(t p) n -> p t n", p=128),
                        in_=state["ost"])

                units = [start]
                seqs = []
                for h in range(HPC):
                    seqs.append(lambda h=h: head_a(h))
                    seqs.append(lambda h=h: head_b(h))
                # reorder: a0, a1, b0, a2, b1, ... (1-head lag)
                order = []
                for h in range(HPC):
                    order.append(seqs[2 * h])
                    if h >= 1:
                        order.append(seqs[2 * (h - 1) + 1])
                order.append(seqs[2 * (HPC - 1) + 1])
                units.extend(order)
                units.append(finish)
                return units

            def interleave(a, b):
                """Merge unit lists proportionally (a paced against b)."""
                if not b:
                    return list(a)
                out = []
                na, nb_ = len(a), len(b)
                ia = ib_ = 0
                while ia < na or ib_ < nb_:
                    if ib_ * na <= ia * nb_:
                        if ib_ < nb_:
                            out.append(b[ib_]); ib_ += 1
                        else:
                            out.append(a[ia]); ia += 1
                    else:
                        if ia < na:
                            out.append(a[ia]); ia += 1
                        else:
                            out.append(b[ib_]); ib_ += 1
                return out

            for sc in range(NCH):
                p1 = phase1_units(sc)
                att = attn_units(sc - 1) if sc > 0 else []
                p1[0]()
                for u in interleave(p1[1:], att):
                    u()
            for u in attn_units(NCH - 1):
                u()
    nc.compile()
    return nc


def _prep_inputs(x, char_ids, Wqkv, bqkv):
    x = np.asarray(x, dtype=np.float32)
    Wqkv = np.asarray(Wqkv, dtype=np.float32)
    bqkv = np.asarray(bqkv, dtype=np.float32)
    seg, starts, ends = _segments(char_ids)
    blocks, maskc = _geometry(seg, starts, ends)
    masks = [_masks_for_batch(seg[b], blocks, maskc) for b in range(B)]

    bf = ml_dtypes.bfloat16
    ident = np.eye(128, dtype=np.float32)
    sq = np.float32(1.0 / np.sqrt(D))
    in_maps = []
    for core in range(NCORES):
        b, hh = core // 2, core % 2
        c0 = hh * CH
        wq = Wqkv[:, c0:c0 + CH] * sq
        wk = Wqkv[:, E + c0:E + c0 + CH]
        wv = Wqkv[:, 2 * E + c0:2 * E + c0 + CH] * np.float32(1.0 / D)
        bq = bqkv[c0:c0 + CH] * sq
        bk = bqkv[E + c0:E + c0 + CH]
        bv = bqkv[2 * E + c0:2 * E + c0 + CH] * np.float32(1.0 / D)
        w = np.ascontiguousarray(
            np.concatenate([wq, wk, wv], axis=1)).astype(bf)
        bqk = np.ascontiguousarray(
            np.concatenate([bq.reshape(4, 128).T, bk.reshape(4, 128).T], axis=1))
        in_maps.append({
            "x": np.ascontiguousarray(x[b]).astype(bf),
            "w": w,
            "bqk": bqk,
            "bv": np.ascontiguousarray(bv.reshape(1, CH)).astype(bf),
            "masks": masks[b],
            "ident": ident,
        })
    return in_maps, blocks, maskc


def kernel(x, char_ids, Wqkv, bqkv):
    from concourse.bass_utils import run_bass_kernel_spmd

    in_maps, blocks, maskc = _prep_inputs(x, char_ids, Wqkv, bqkv)
    key = repr((tuple(tuple(b) for b in blocks), maskc))
    if key not in _prog_cache:
        _prog_cache[key] = _build_program(blocks, maskc)
    nc = _prog_cache[key]

    res = run_bass_kernel_spmd(nc, in_maps, list(range(NCORES)))
    out = np.empty((B, S, E), dtype=np.float32)
    for core in range(NCORES):
        b, hh = core // 2, core % 2
        out[b, :, hh * CH:(hh + 1) * CH] = res.results[core]["out"]
    return out


# revision 15
# speedup vs baseline: 1.0401x; 1.0401x over previous
"""Character-delimited (segment-local causal) attention on 8 trn2 cores.

Sharding: core = (batch, head-half): b = core//2, hh = core%2.
Each core computes the qkv projection for its batch restricted to its 8
heads (512 of the 3072 Wqkv columns per section) plus the segment-sparse
attention for those heads.

Device pipeline (per core, bf16 matmul operands / fp32 accumulation):
  - x is loaded TRANSPOSED via xbar DMA-transpose (bf16) as xT tiles.
  - q,k are produced transposed (qT/kT: [d, s]) so QK^T needs no further
    transposes; v is produced in natural [s, d] layout with an extra
    all-ones column per head so the PV matmul also accumulates the softmax
    denominator (row 64 of the ctx psum).
  - Attention is block-sparse: for each 512-query chunk only key blocks
    that can contain same-segment keys are computed (block geometry is
    derived on the host from char_ids and baked into the program; per-batch
    exactness is restored by elementwise 0/1 masks multiplied into
    exp(scores)).  scores are computed transposed ([k, q]) so exp(scores)
    feeds the PV matmul directly as the moving operand.
  - ctx^T [65, 512] (64 dims + denominator row) is transposed back on the
    PE (fp32) and the division by the denominator is folded into the
    PSUM->SBUF copy via a per-partition reciprocal scale.
"""

import numpy as np
import ml_dtypes

B, S, E = 4, 2048, 1024
H, D = 16, 64
NCORES = 8
CH = 512          # query chunk
KB = 128          # key block
NCH = S // CH     # 4 chunks
DELIMS = (32, 10)
HPC = H // 2      # heads per core (8)

_prog_cache = {}


def _segments(char_ids):
    """seg ids, per-position segment start / end (exclusive), per batch."""
    ids = np.asarray(char_ids)
    is_d = np.zeros(ids.shape, dtype=bool)
    for d in DELIMS:
        is_d |= ids == d
    seg = np.cumsum(is_d.astype(np.int64), axis=-1)
    starts = np.empty_like(seg)
    ends = np.empty_like(seg)
    for b in range(seg.shape[0]):
        starts[b] = np.searchsorted(seg[b], seg[b], side="left")
        ends[b] = np.searchsorted(seg[b], seg[b], side="right")
    return seg, starts, ends


def _geometry(seg, starts, ends):
    """Shared (union over batches) block geometry.

    Returns blocks[qc] = list of (k0, qoff, N, moff) and mask width MASKC.
    """
    blocks = []
    maskc = 0
    for qc in range(NCH):
        q0 = qc * CH
        sstart_min = int(starts[:, q0].min())
        send_max = int(ends[:, q0].max())
        past_lo = (sstart_min // KB) * KB
        qe_past = min(send_max - q0, CH)
        blist = []
        moff = 0
        for k0 in range(past_lo, q0, KB):
            blist.append((k0, 0, qe_past, moff))
            moff += qe_past
        for kc in range(CH // KB):
            k0 = q0 + kc * KB
            de = int(ends[:, k0 + KB - 1].max())
            de = min(max(de, k0 + KB), q0 + CH)
            n = de - k0
            blist.append((k0, kc * KB, n, moff))
            moff += n
        blocks.append(blist)
        maskc = max(maskc, moff)
    return blocks, maskc


def _masks_for_batch(seg_b, blocks, maskc):
    """[NCH, 128, maskc] bfloat16 0/1 mask blob for one batch."""
    out = np.zeros((NCH, KB, maskc), dtype=ml_dtypes.bfloat16)
    pos = np.arange(S)
    for qc, blist in enumerate(blocks):
        q0 = qc * CH
        for (k0, qoff, n, moff) in blist:
            kk = pos[k0:k0 + KB]
            qq = pos[q0 + qoff:q0 + qoff + n]
            m = (seg_b[kk][:, None] == seg_b[qq][None, :]) & (kk[:, None] <= qq[None, :])
            out[qc, :, moff:moff + n] = m.astype(ml_dtypes.bfloat16)
    return out


def _group_blocks(blist):
    """Pack consecutive blocks into groups whose total q-extent fits one
    512-col psum bank.  Returns [(g_moff, gN, [(k0, qoff, n, moff), ...])]."""
    groups = []
    cur, width = [], 0
    for blk in blist:
        n = blk[2]
        if cur and width + n > CH:
            groups.append((cur[0][3], width, cur))
            cur, width = [], 0
        cur.append(blk)
        width += n
    if cur:
        groups.append((cur[0][3], width, cur))
    return groups


def _build_program(blocks, maskc):
    import concourse.bacc as bacc
    import concourse.tile as tile
    from concourse import mybir
    from contextlib import ExitStack

    f32 = mybir.dt.float32
    bf16 = mybir.dt.bfloat16
    AF = mybir.ActivationFunctionType

    nc = bacc.Bacc("TRN2", target_bir_lowering=False, debug=False,
                   num_devices=NCORES)

    x_h = nc.dram_tensor("x", [S, E], bf16, kind="ExternalInput")
    w_h = nc.dram_tensor("w", [E, 3 * CH], bf16, kind="ExternalInput")
    bqk_h = nc.dram_tensor("bqk", [128, 8], f32, kind="ExternalInput")
    bv_h = nc.dram_tensor("bv", [1, CH], bf16, kind="ExternalInput")
    mk_h = nc.dram_tensor("masks", [NCH, KB, maskc], bf16, kind="ExternalInput")
    id_h = nc.dram_tensor("ident", [128, 128], f32, kind="ExternalInput")
    out_h = nc.dram_tensor("out", [S, CH], f32, kind="ExternalOutput")

    ET = E // 128   # 8 e-tiles

    with tile.TileContext(nc) as tc:
        with ExitStack() as ctx:
            sing = ctx.enter_context(tc.tile_pool(name="sing", bufs=1))
            xtp = ctx.enter_context(tc.tile_pool(name="xtp", bufs=12))
            qp = ctx.enter_context(tc.tile_pool(name="qp", bufs=2))
            mp = ctx.enter_context(tc.tile_pool(name="mp", bufs=2))
            esp = ctx.enter_context(tc.tile_pool(name="esp", bufs=7))
            ctsp = ctx.enter_context(tc.tile_pool(name="ctsp", bufs=3))
            recp = ctx.enter_context(tc.tile_pool(name="recp", bufs=3))
            outp = ctx.enter_context(tc.tile_pool(name="outp", bufs=2))

            ph1 = ctx.enter_context(tc.tile_pool(name="ph1", bufs=2, space="PSUM"))
            scrp = ctx.enter_context(tc.tile_pool(name="scrp", bufs=3, space="PSUM"))
            ctxp = ctx.enter_context(tc.tile_pool(name="ctxp", bufs=3, space="PSUM"))

            # ---- constants / persistent tensors ----
            w_sb = sing.tile([128, ET, 3 * CH], bf16, tag="w")
            for et in range(ET):
                nc.scalar.dma_start(out=w_sb[:, et, :],
                                    in_=w_h[et * 128:(et + 1) * 128, :])
            id_sb = sing.tile([128, 128], f32, tag="ident")
            nc.sync.dma_start(out=id_sb, in_=id_h[:, :])
            bqk_sb = sing.tile([128, 8], f32, tag="bqk")
            nc.sync.dma_start(out=bqk_sb, in_=bqk_h[:, :])
            bv_sb = sing.tile([1, CH], bf16, tag="bv")
            nc.sync.dma_start(out=bv_sb, in_=bv_h[:, :])
            ones_sb = sing.tile([1, 128], bf16, tag="ones")
            nc.vector.memset(ones_sb, 1.0)
            zcol_sb = sing.tile([1, 65], bf16, tag="zcol")
            nc.vector.memset(zcol_sb, 0.0)

            k_sbs, v_sbs = [], []
            for c in range(NCH):
                kt_ = sing.tile([128, 4, CH], bf16, tag=f"k{c}")
                vt_ = sing.tile([128, 4, HPC, 65], bf16, tag=f"v{c}")
                nc.vector.memset(vt_[:, :, :, 64:65], 1.0)
                k_sbs.append(kt_)
                v_sbs.append(vt_)

            # -------- software-pipelined emission: phase1(sc) ∥ attn(sc-1) ----
            q_tiles = {}
            mask_tiles = {}

            def phase1_units(sc):
                units = []

                def load(sc=sc):
                    xts = []
                    if sc == 0:
                        # chunk 0: PE transposes (x arrives via fast plain
                        # DMAs; the PE is idle at startup anyway) to avoid
                        # the serialized xbar-transpose descriptor latency
                        xrows = []
                        for ss in range(4):
                            xr = xtp.tile([128, E], bf16, tag="xr", name="xr")
                            nc.sync.dma_start(
                                out=xr, in_=x_h[ss * 128:(ss + 1) * 128, :])
                            xrows.append(xr)
                        idb = sing.tile([128, 128], bf16, tag="idb")
                        nc.vector.tensor_copy(idb, id_sb)
                        for et in range(ET):
                            pt = scrp.tile([128, CH], bf16, tag="scr",
                                           name="pt")
                            for ss in range(4):
                                nc.tensor.transpose(
                                    pt[:, ss * 128:(ss + 1) * 128],
                                    xrows[ss][:, et * 128:(et + 1) * 128],
                                    idb)
                            xt_t = xtp.tile([128, CH], bf16, tag="xt")
                            if et % 2 == 0:
                                nc.scalar.copy(xt_t, pt)
                            else:
                                nc.vector.tensor_copy(xt_t, pt)
                            xts.append(xt_t)
                    else:
                        for et in range(ET):
                            xt_t = xtp.tile([128, CH], bf16, tag="xt")
                            nc.sync.dma_start(
                                out=xt_t,
                                in_=x_h[sc * CH:(sc + 1) * CH,
                                        et * 128:(et + 1) * 128],
                                transpose=True)
                            xts.append(xt_t)
                    phase1_units.xts = xts
                    mask_t = mp.tile([128, maskc], bf16, tag="m")
                    nc.sync.dma_start(out=mask_t, in_=mk_h[sc, :, :])
                    mask_tiles[sc] = mask_t
                    q_tiles[sc] = qp.tile([128, 4, CH], bf16, tag="q", name="q_t")
                units.append(load)

                def qk_unit(ot, sc=sc):
                    xts = phase1_units.xts
                    pq = ph1.tile([128, CH], f32, tag="ph1")
                    for et in range(ET):
                        nc.tensor.matmul(
                            pq, w_sb[:, et, ot * 128:(ot + 1) * 128], xts[et],
                            start=(et == 0), stop=(et == ET - 1))
                    if ot < 4:
                        nc.scalar.add(q_tiles[sc][:, ot, :], pq,
                                      bqk_sb[:, ot:ot + 1])
                    else:
                        nc.vector.tensor_scalar_add(k_sbs[sc][:, ot - 4, :],
                                                    pq, bqk_sb[:, ot:ot + 1])
                for ot in range(8):
                    units.append(lambda ot=ot: qk_unit(ot))

                def v_unit(ss, sc=sc):
                    xts = phase1_units.xts
                    pv = ph1.tile([128, CH], f32, tag="ph1")
                    for et in range(ET):
                        nc.tensor.matmul(
                            pv, xts[et][:, ss * 128:(ss + 1) * 128],
                            w_sb[:, et, 2 * CH:3 * CH],
                            start=(et == 0), stop=False)
                    nc.tensor.matmul(pv, ones_sb, bv_sb, start=False, stop=True)
                    nc.vector.tensor_copy(
                        v_sbs[sc][:, ss, :, 0:64],
                        pv.rearrange("p (h c) -> p h c", c=64))
                for ss in range(4):
                    units.append(lambda ss=ss: v_unit(ss))
                return units

            def attn_units(qc):
                groups = _group_blocks(blocks[qc])
                nb = len(blocks[qc])
                state = {}

                def start(qc=qc):
                    state["ost"] = outp.tile([128, 4, CH], f32, tag="ost", name="ost")

                def head_a(h, qc=qc):
                    q_t = q_tiles[qc]
                    mask_t = mask_tiles[qc]
                    p0 = (h % 2) * 64
                    ctx_t = ctxp.tile([65, CH], f32, tag="ct", name="ctx_t")
                    nc.tensor.matmul(ctx_t, zcol_sb, bv_sb,
                                     start=True, stop=False)
                    scrs = []
                    for gi2, (gm, gn, blks) in enumerate(groups):
                        if qc == NCH - 1 and gi2 % 2 == 0:
                            scr = ph1.tile([128, CH], f32, tag="ph1", name="scr")
                        else:
                            scr = scrp.tile([128, CH], f32, tag="scr")
                        for (k0, qoff, n, moff) in blks:
                            kci, koff = k0 // CH, k0 % CH
                            nc.tensor.matmul(
                                scr[:, moff - gm:moff - gm + n],
                                k_sbs[kci][p0:p0 + 64, h // 2,
                                           koff:koff + 128],
                                q_t[p0:p0 + 64, h // 2, qoff:qoff + n],
                                start=True, stop=True)
                        scrs.append(scr)
                    ess = []
                    for gi, (gm, gn, blks) in enumerate(groups):
                        es = esp.tile([128, CH], bf16, tag="es")
                        nc.scalar.activation(es[:, 0:gn], scrs[gi][:, 0:gn],
                                             AF.Exp)
                        nc.vector.tensor_mul(es[:, 0:gn], es[:, 0:gn],
                                             mask_t[:, gm:gm + gn])
                        ess.append(es)
                    state[("h", h)] = (ctx_t, ess)

                def head_b(h, qc=qc):
                    ost = state["ost"]
                    ctx_t, ess = state.pop(("h", h))
                    nb = len(blocks[qc])
                    ib = 0
                    for gi, (gm, gn, blks) in enumerate(groups):
                        for (k0, qoff, n, moff) in blks:
                            kci, koff = k0 // CH, k0 % CH
                            nc.tensor.matmul(
                                ctx_t[:, qoff:qoff + n],
                                v_sbs[kci][:, koff // 128, h, :],
                                ess[gi][:, moff - gm:moff - gm + n],
                                start=False, stop=(ib == nb - 1))
                            ib += 1

                    cts = ctsp.tile([65, CH], f32, tag="cts")
                    if h % 2 == 0:
                        nc.vector.tensor_copy(cts, ctx_t)
                    else:
                        nc.scalar.copy(cts, ctx_t)
                    tp = ctxp.tile([128, 4 * 65], f32, tag="ct", name="tp")
                    for i in range(4):
                        nc.tensor.transpose(
                            tp[:, i * 65:(i + 1) * 65],
                            cts[:, i * 128:(i + 1) * 128],
                            id_sb[0:65, 0:65])
                    rec = recp.tile([128, 4, 1], f32, tag="rec")
                    nc.vector.reciprocal(
                        rec,
                        tp.rearrange("p (i c) -> p i c", c=65)[:, :, 64:65])
                    for i in range(4):
                        if h % 2 == 0:
                            nc.scalar.mul(ost[:, i, h * 64:(h + 1) * 64],
                                          tp[:, i * 65:i * 65 + 64],
                                          rec[:, i:i + 1, 0])
                        else:
                            nc.vector.tensor_scalar_mul(
                                ost[:, i, h * 64:(h + 1) * 64],
                                tp[:, i * 65:i * 65 + 64],
                                rec[:, i:i + 1, 0])

                def finish(qc=qc):
                    nc.scalar.dma_start(
                        out=out_h[qc * CH:(qc + 1) * CH, :].rearrange(
                            "(t p) n -> p t n", p=128),
                        in_=state["ost"])

                units = [start]
                seqs = []
                for h in range(HPC):
                    seqs.append(lambda h=h: head_a(h))
                    seqs.append(lambda h=h: head_b(h))
                # reorder: a0, a1, b0, a2, b1, ... (1-head lag)
                order = []
                for h in range(HPC):
                    order.append(seqs[2 * h])
                    if h >= 1:
                        order.append(seqs[2 * (h - 1) + 1])
                order.append(seqs[2 * (HPC - 1) + 1])
                units.extend(order)
                units.append(finish)
                return units

            def interleave(a, b):
                """Merge unit lists proportionally (a paced against b)."""
                if not b:
                    return list(a)
                out = []
                na, nb_ = len(a), len(b)
                ia = ib_ = 0
                while ia < na or ib_ < nb_:
                    if ib_ * na <= ia * nb_:
                        if ib_ < nb_:
                            out.append(b[ib_]); ib_ += 1
                        else:
                            out.append(a[ia]); ia += 1
                    else:
                        if ia < na:
                            out.append(a[ia]); ia += 1
                        else:
                            out.append(b[ib_]); ib_ += 1
                return out

            for sc in range(NCH):
                p1 = phase1_units(sc)
                att = attn_units(sc - 1) if sc > 0 else []
                p1[0]()
                for u in interleave(p1[1:], att):
                    u()
            for u in attn_units(NCH - 1):
                u()
    nc.compile()
    return nc


def _prep_inputs(x, char_ids, Wqkv, bqkv):
    x = np.asarray(x, dtype=np.float32)
    Wqkv = np.asarray(Wqkv, dtype=np.float32)
    bqkv = np.asarray(bqkv, dtype=np.float32)
    seg, starts, ends = _segments(char_ids)
    blocks, maskc = _geometry(seg, starts, ends)
    masks = [_masks_for_batch(seg[b], blocks, maskc) for b in range(B)]

    bf = ml_dtypes.bfloat16
    ident = np.eye(128, dtype=np.float32)
    sq = np.float32(1.0 / np.sqrt(D))
    in_maps = []
    for core in range(NCORES):
        b, hh = core // 2, core % 2
        c0 = hh * CH
        wq = Wqkv[:, c0:c0 + CH] * sq
        wk = Wqkv[:, E + c0:E + c0 + CH]
        wv = Wqkv[:, 2 * E + c0:2 * E + c0 + CH] * np.float32(1.0 / D)
        bq = bqkv[c0:c0 + CH] * sq
        bk = bqkv[E + c0:E + c0 + CH]
        bv = bqkv[2 * E + c0:2 * E + c0 + CH] * np.float32(1.0 / D)
        w = np.ascontiguousarray(
            np.concatenate([wq, wk, wv], axis=1)).astype(bf)
        bqk = np.ascontiguousarray(
            np.concatenate([bq.reshape(4, 128).T, bk.reshape(4, 128).T], axis=1))
        in_maps.append({
            "x": np.ascontiguousarray(x[b]).astype(bf),
            "w": w,
            "bqk": bqk,
            "bv": np.ascontiguousarray(bv.reshape(1, CH)).astype(bf),
            "masks": masks[b],
            "ident": ident,
        })
    return in_maps, blocks, maskc


def kernel(x, char_ids, Wqkv, bqkv):
    from concourse.bass_utils import run_bass_kernel_spmd

    in_maps, blocks, maskc = _prep_inputs(x, char_ids, Wqkv, bqkv)
    key = repr((tuple(tuple(b) for b in blocks), maskc))
    if key not in _prog_cache:
        _prog_cache[key] = _build_program(blocks, maskc)
    nc = _prog_cache[key]

    res = run_bass_kernel_spmd(nc, in_maps, list(range(NCORES)))
    out = np.empty((B, S, E), dtype=np.float32)
    for core in range(NCORES):
        b, hh = core // 2, core % 2
        out[b, :, hh * CH:(hh + 1) * CH] = res.results[core]["out"]
    return out
